# revision 20
# baseline (speedup 1.0000x reference)
"""MCR loss kernel for Trainium2 (8 NeuronCores), v3.

Per core: 2 timesteps x 3 feature maps = 6 input planes [32c, 192h, 192w].

  - DMA: 24-row h-slabs, pass A = planes 0-3 on 128 partitions (g,c),
    pass B = planes 4-5 on 64 partitions; 18.4 KB contiguous runs per
    partition, interleaved A/B so the DVE is continuously fed.
  - stage 1 (w-direction 8:1 pool) on DVE: tensor_reduce over the
    contiguous innermost 8, writing an x-major transposed intermediate
    [p, (x24, h24)] so stage 2 also reduces a contiguous axis.
  - stage 2 (h-direction 8:1) on DVE: reduce over r8 (contiguous),
    writing bf16 directly into a reflect-padded x-major conv input
    [p, 26x, 26y]; 4 small edge copies per pad finish the pad.
  - conv: 9 shifted bf16 matmuls (1 cyc/col) with block-diag [96,96]
    stationaries for t0; 27 row/col-tiled matmuls for t1 (its planes
    straddle the two pad buffers). LeakyReLU(0.2) = Act copy + DVE
    max(0.2z, z), output V in bf16 (x-major pixel order; the Gram is
    invariant to pixel order).
  - Gram G_t = V_t V_t^T via bf16 PE transpose + matmul chunks.
  - Host: logdet(I_576 + a V^T V) = logdet(I_96 + a V V^T); float64
    Cholesky on [16,96,96] Grams finishes the scalar loss.
"""

import numpy as np

_STATE = {}

# -------- fixed problem geometry (hardcoded per harness contract) --------
B, CCH, H, W = 16, 32, 192, 192
NCORES = 8
TPC = B // NCORES          # timesteps per core = 2
OUT = 24                   # pooled spatial size
PIX = OUT * OUT            # 576
M = 96                     # feature rows (3 maps x 32 channels)
ALPHA_E = 6.0              # 576 / (96 * eps)
ALPHA_C = 18.0             # 576 / (32 * eps)
PAD = 26                   # padded conv input edge
PPIX = PAD * PAD           # 676
NQ = 8                     # 24-row h-slabs per pass
HR = H // NQ               # 24 rows per slab


def _build_nc():
    import concourse.bass as bass
    import concourse.tile as tile
    from concourse import bacc, mybir

    DT = mybir.dt.float32
    BF = mybir.dt.bfloat16

    nc = bacc.Bacc(
        "TRN2", target_bir_lowering=False, debug=False, num_devices=NCORES
    )

    # x[g] for g = t*3+m : feature-map plane stacks, host-reordered
    x = nc.declare_dram_parameter("x", [TPC * 3, CCH, H, W], DT, isOutput=False)
    # block-diag conv weights: wt[(m,ic), (dy*3+dx)*96 + (m,oc)], bf16
    wt = nc.declare_dram_parameter("wt", [96, 9 * 96], BF, isOutput=False)
    # t1 weights at partitions matching their fmap: m1@0, m2@32, m0@96
    wtt1 = nc.declare_dram_parameter("wtt1", [128, 9 * 64], BF, isOutput=False)
    ident = nc.declare_dram_parameter("ident", [96, 96], BF, isOutput=False)
    g_out = nc.declare_dram_parameter("g_out", [TPC, M, M], DT, isOutput=True)

    with tile.TileContext(nc) as tc:
        with (
            tc.tile_pool(name="persist", bufs=1) as persist,
            tc.tile_pool(name="slabsA", bufs=3) as slabsA,
            tc.tile_pool(name="slabsB", bufs=3) as slabsB,
            tc.tile_pool(name="wsums", bufs=2) as wsumsA,
            tc.tile_pool(name="wsumsB", bufs=2) as wsumsB,
            tc.tile_pool(name="vt", bufs=2) as vtpool,
            tc.tile_pool(name="convps", bufs=2, space="PSUM") as convps,
            tc.tile_pool(name="vtps", bufs=2, space="PSUM") as vtps,
            tc.tile_pool(name="gramps", bufs=1, space="PSUM") as gramps,
        ):
            wt_sb = persist.tile([96, 9 * 96], BF, tag="wt")
            nc.gpsimd.dma_start(out=wt_sb[:], in_=wt.ap())
            wtt1_sb = persist.tile([128, 9 * 64], BF, tag="wtt1")
            nc.gpsimd.dma_start(out=wtt1_sb[:], in_=wtt1.ap())
            id_sb = persist.tile([96, 96], BF, tag="ident")
            nc.gpsimd.dma_start(out=id_sb[:], in_=ident.ap())

            # x-major reflect-padded pooled conv inputs (bf16)
            padA = persist.tile([128, PPIX], BF, tag="padA")  # planes 0-3
            padB = persist.tile([64, PPIX], BF, tag="padB")   # planes 4-5
            v_sb = persist.tile([96, TPC * PIX], BF, tag="v")
            g_sb = persist.tile([96, TPC * 96], DT, tag="g")

            pA3 = padA[:].rearrange("p (x y) -> p x y", x=PAD)
            pB3 = padB[:].rearrange("p (x y) -> p x y", x=PAD)

            # ---- pooling helpers ----
            def slab_dma(pas, q):
                gl, np_ = (0, 128) if pas == "A" else (4, 64)
                spool = slabsA if pas == "A" else slabsB
                slab = spool.tile([np_, HR * W], DT, tag=f"slab{pas}")
                nc.sync.dma_start(
                    out=slab[:],
                    in_=x.ap()[
                        gl : gl + np_ // 32, :, HR * q : HR * (q + 1), :
                    ].rearrange("g c h w -> (g c) (h w)"),
                )
                return slab

            def slab_reduce(pas, q, slab):
                np_ = 128 if pas == "A" else 64
                pad3 = pA3 if pas == "A" else pB3
                wpool = wsumsA if pas == "A" else wsumsB
                # stage 1: w-pool 8:1, contiguous inner reduce, x-major dst
                wsum = wpool.tile([np_, HR * OUT], DT, tag=f"ws{pas}")
                nc.vector.tensor_reduce(
                    out=wsum[:].rearrange("p (x h) -> p h x", h=HR),
                    in_=slab[:].rearrange("p (h x b) -> p h x b", x=OUT, b=8),
                    axis=mybir.AxisListType.X,
                    op=mybir.AluOpType.add,
                )
                # stage 2: h-pool 8:1, bf16 out into padded interior rows
                with nc.allow_low_precision(
                    reason="pooled conv input in bf16 (tol 2e-2)"
                ):
                    nc.vector.tensor_reduce(
                        out=pad3[:np_, 1:25, 3 * q + 1 : 3 * q + 4],
                        in_=wsum[:].rearrange(
                            "p (x hg r) -> p x hg r", hg=HR // 8, r=8
                        ),
                        axis=mybir.AxisListType.X,
                        op=mybir.AluOpType.add,
                    )

            def fix_pad0(pad3):
                # reflect pad, part 1 (after slab q4): x-edge rows for
                # y-cols 1..13, then y-col 0 over all x
                nc.vector.tensor_copy(pad3[:, 0:1, 1:14], pad3[:, 2:3, 1:14])
                nc.vector.tensor_copy(pad3[:, 25:26, 1:14], pad3[:, 23:24, 1:14])
                nc.vector.tensor_copy(pad3[:, :, 0:1], pad3[:, :, 2:3])

            def fix_pad1(pad3):
                # reflect pad, part 2 (after slab q7): remaining x-edge
                # cols, then y-col 25 over all x
                nc.vector.tensor_copy(pad3[:, 0:1, 14:25], pad3[:, 2:3, 14:25])
                nc.vector.tensor_copy(pad3[:, 25:26, 14:25], pad3[:, 23:24, 14:25])
                nc.vector.tensor_copy(pad3[:, :, 25:26], pad3[:, :, 23:24])

            dydx = [(a, b) for a in range(3) for b in range(3)]

            def conv_t_yh(t, yh):
                # one y-half of the conv; V pixel order is y-major so each
                # half fills a contiguous 288-col V slice
                pcb = convps.tile([96, 512], DT, tag="convps")
                pc = pcb[:, 0:288]
                if t == 0:
                    for i, (dy, dx) in enumerate(dydx):
                        blk = dy * 3 + dx
                        nc.tensor.matmul(
                            pc[:],
                            wt_sb[:, blk * 96 : (blk + 1) * 96],
                            pA3[:96, dx : dx + 24,
                                yh * 12 + dy : yh * 12 + dy + 12
                                ].transpose([0, 2, 1]),
                            start=(i == 0), stop=(i == 8),
                        )
                else:
                    # t1: V rows (m1, m2, m0); m1+m2 fused 64-wide on padB,
                    # m0 from padA[96:128] at position (96, 64); the host
                    # permutes G[t1] back afterwards
                    for i, (dy, dx) in enumerate(dydx):
                        blk = dy * 3 + dx
                        nc.tensor.matmul(
                            pc[0:64, :],
                            wtt1_sb[0:64, blk * 64 : blk * 64 + 64],
                            pB3[:, dx : dx + 24,
                                yh * 12 + dy : yh * 12 + dy + 12
                                ].transpose([0, 2, 1]),
                            start=(i == 0), stop=(i == 8),
                        )
                    for i, (dy, dx) in enumerate(dydx):
                        blk = dy * 3 + dx
                        nc.tensor.matmul(
                            pc[64:96, :],
                            wtt1_sb[96:128, blk * 64 : blk * 64 + 32],
                            pA3[96:128, dx : dx + 24,
                                yh * 12 + dy : yh * 12 + dy + 12
                                ].transpose([0, 2, 1]),
                            start=(i == 0), stop=(i == 8),
                            tile_position=(96, 64),
                        )
                # LeakyReLU(0.2) == max(0.2*z, z); PSUM feeds only one
                # non-scalar input, so stage a copy through SBUF
                zc = vtpool.tile([96, 288], DT, tag="zcopy")
                nc.scalar.copy(zc[:], pc[:])
                nc.vector.scalar_tensor_tensor(
                    out=v_sb[:, t * PIX + yh * 288 : t * PIX + (yh + 1) * 288],
                    in0=zc[:],
                    scalar=0.2,
                    in1=pc[:],
                    op0=mybir.AluOpType.mult,
                    op1=mybir.AluOpType.max,
                )

            vt_alls = {}

            def gram_chunks(t, chunks):
                if t not in vt_alls:
                    vt_all = vtpool.tile([128, 5 * 96], BF, tag=f"vtall{t}")
                    vt_alls[t] = vt_all
                vt_all = vt_alls[t]
                for c in chunks:
                    sz = 128 if c < 4 else 64
                    vslice = v_sb[:, t * PIX + c * 128 : t * PIX + c * 128 + sz]
                    ptb = vtps.tile([128, 1024], BF, tag="vtps")
                    pt = ptb[:, 0:96]
                    nc.tensor.transpose(pt[:sz, :], vslice, id_sb[:])
                    nc.scalar.copy(vt_all[:sz, c * 96 : (c + 1) * 96], pt[:sz, :])

            def gram_finish(t):
                vt_all = vt_alls[t]
                gpb = gramps.tile([96, 512], DT, tag="gram")
                gp = gpb[:, 0:96]
                for c in range(5):
                    sz = 128 if c < 4 else 64
                    nc.tensor.matmul(
                        gp[:],
                        vt_all[:sz, c * 96 : (c + 1) * 96],
                        vt_all[:sz, c * 96 : (c + 1) * 96],
                        start=(c == 0), stop=(c == 4),
                    )
                nc.scalar.copy(g_sb[:, t * 96 : (t + 1) * 96], gp[:])
                nc.gpsimd.dma_start(
                    out=g_out[t], in_=g_sb[:, t * 96 : (t + 1) * 96]
                )

            # ---- schedule: B0 first (fast DVE start); conv/relu/gram
            # y-halves stream in as soon as their pad rows complete ----
            for step in ["B0", "A0", "A1", "B1", "A2", "B2", "A3", "B3",
                         "A4", "W0", "B4", "A5", "B5", "W1", "A6", "A7",
                         "W2", "B6", "B7", "W3"]:
                if step == "W0":
                    fix_pad0(pA3)
                    conv_t_yh(0, 0)
                    gram_chunks(0, [0, 1])
                elif step == "W1":
                    fix_pad0(pB3)
                    conv_t_yh(1, 0)
                    gram_chunks(1, [0, 1])
                elif step == "W2":
                    fix_pad1(pA3)
                    conv_t_yh(0, 1)
                    gram_chunks(0, [2, 3, 4])
                    gram_finish(0)
                elif step == "W3":
                    fix_pad1(pB3)
                    conv_t_yh(1, 1)
                    gram_chunks(1, [2, 3, 4])
                    gram_finish(1)
                else:
                    pas, q = step[0], int(step[1])
                    slab_reduce(pas, q, slab_dma(pas, q))

    nc.finalize()
    return nc


def _get_nc():
    if "nc" not in _STATE:
        _STATE["nc"] = _build_nc()
    return _STATE["nc"]


def _prep_weights(W1, W2, W3):
    import ml_dtypes

    # wt[(m,ic), (dy*3+dx)*96 + 32m+oc] = W_m[oc, ic, dy, dx] / 64
    wt = np.zeros((96, 9 * 96), dtype=np.float64)
    for m, Wm in enumerate((W1, W2, W3)):
        Wm = np.asarray(Wm, np.float64) / 64.0  # [oc, ic, dy, dx]
        for dy in range(3):
            for dx in range(3):
                blk = dy * 3 + dx
                wt[
                    32 * m : 32 * m + 32,
                    blk * 96 + 32 * m : blk * 96 + 32 * m + 32,
                ] = Wm[:, :, dy, dx].T
    # wtt1 for t1 (V rows ordered m1, m2, m0): rows 0-31 = W2 ic (oc cols
    # 0-31), rows 32-63 = W3 ic (oc cols 32-63), rows 96-127 = W1 ic (oc
    # cols 0-31 of its own [32,32] slice) -- partition-aligned with fmaps
    wtt1 = np.zeros((128, 9 * 64), dtype=np.float64)
    for dy in range(3):
        for dx in range(3):
            blk = dy * 3 + dx
            w2 = np.asarray(W2, np.float64) / 64.0
            w3 = np.asarray(W3, np.float64) / 64.0
            w1 = np.asarray(W1, np.float64) / 64.0
            wtt1[0:32, blk * 64 : blk * 64 + 32] = w2[:, :, dy, dx].T
            wtt1[32:64, blk * 64 + 32 : blk * 64 + 64] = w3[:, :, dy, dx].T
            wtt1[96:128, blk * 64 : blk * 64 + 32] = w1[:, :, dy, dx].T
    bf = ml_dtypes.bfloat16
    return wt.astype(np.float32).astype(bf), wtt1.astype(np.float32).astype(bf)


def _host_loss(G):
    G = np.asarray(G, np.float64)  # [16, 96, 96]
    T = G.shape[0]
    I96 = np.eye(M)
    Me = I96[None] + ALPHA_E * G
    ld_e = 2.0 * np.log(
        np.diagonal(np.linalg.cholesky(Me), axis1=-2, axis2=-1)
    ).sum()
    blocks = np.stack(
        [G[:, 32 * c : 32 * (c + 1), 32 * c : 32 * (c + 1)] for c in range(3)]
    )  # [3, T, 32, 32]
    Mc = np.eye(32)[None, None] + ALPHA_C * blocks
    ld_c = 2.0 * np.log(
        np.diagonal(np.linalg.cholesky(Mc), axis1=-2, axis2=-1)
    ).sum()
    loss_expd = ld_e / (2.0 * T)
    loss_comp = (32.0 / M) * ld_c / (2.0 * T)
    return np.float32(loss_expd - loss_comp)


def run_device(inputs, **kw):
    """Run the bass kernel; returns (G [16,96,96], BassKernelResults)."""
    import ml_dtypes
    from concourse.bass_utils import run_bass_kernel_spmd

    nc = _get_nc()
    wt, wtt1 = _prep_weights(inputs["W1"], inputs["W2"], inputs["W3"])
    ident = np.eye(96, dtype=np.float32).astype(ml_dtypes.bfloat16)
    ms = np.asarray(inputs["ms_fea"], np.float32)
    pan = np.asarray(inputs["pan_fea"], np.float32)
    alf = np.asarray(inputs["all_fea"], np.float32)
    in_maps = []
    for i in range(NCORES):
        sl = slice(TPC * i, TPC * (i + 1))
        # x[t*3+m] = (ms,pan,alf)[m][t]
        xs = np.stack([ms[sl], pan[sl], alf[sl]], axis=1).reshape(
            TPC * 3, CCH, H, W
        )
        in_maps.append(
            {"x": np.ascontiguousarray(xs), "wt": wt, "wtt1": wtt1,
             "ident": ident}
        )
    res = run_bass_kernel_spmd(nc, in_maps, core_ids=list(range(NCORES)), **kw)
    G = np.concatenate([np.asarray(r["g_out"]) for r in res.results], axis=0)
    # odd timesteps were computed with V rows ordered (m1, m2, m0)
    perm = np.r_[64:96, 0:32, 32:64]
    G[1::2] = G[1::2][:, perm][:, :, perm]
    return G, res


def kernel(**inputs):
    G, _ = run_device(inputs)
    return _host_loss(G)


# revision 22
# speedup vs baseline: 1.0209x; 1.0209x over previous
"""MCR loss kernel for Trainium2 (8 NeuronCores), v3.

Per core: 2 timesteps x 3 feature maps = 6 input planes [32c, 192h, 192w].

  - DMA: 24-row h-slabs, pass A = planes 0-3 on 128 partitions (g,c),
    pass B = planes 4-5 on 64 partitions; 18.4 KB contiguous runs per
    partition, interleaved A/B so the DVE is continuously fed.
  - stage 1 (w-direction 8:1 pool) on DVE: tensor_reduce over the
    contiguous innermost 8, writing an x-major transposed intermediate
    [p, (x24, h24)] so stage 2 also reduces a contiguous axis.
  - stage 2 (h-direction 8:1) on DVE: reduce over r8 (contiguous),
    writing bf16 directly into a reflect-padded x-major conv input
    [p, 26x, 26y]; 4 small edge copies per pad finish the pad.
  - conv: 9 shifted bf16 matmuls (1 cyc/col) with block-diag [96,96]
    stationaries for t0; 27 row/col-tiled matmuls for t1 (its planes
    straddle the two pad buffers). LeakyReLU(0.2) = Act copy + DVE
    max(0.2z, z), output V in bf16 (x-major pixel order; the Gram is
    invariant to pixel order).
  - Gram G_t = V_t V_t^T via bf16 PE transpose + matmul chunks.
  - Host: logdet(I_576 + a V^T V) = logdet(I_96 + a V V^T); float64
    Cholesky on [16,96,96] Grams finishes the scalar loss.
"""

import numpy as np

_STATE = {}

# -------- fixed problem geometry (hardcoded per harness contract) --------
B, CCH, H, W = 16, 32, 192, 192
NCORES = 8
TPC = B // NCORES          # timesteps per core = 2
OUT = 24                   # pooled spatial size
PIX = OUT * OUT            # 576
M = 96                     # feature rows (3 maps x 32 channels)
ALPHA_E = 6.0              # 576 / (96 * eps)
ALPHA_C = 18.0             # 576 / (32 * eps)
PAD = 26                   # padded conv input edge
PPIX = PAD * PAD           # 676
NQ = 8                     # 24-row h-slabs per pass
HR = H // NQ               # 24 rows per slab


def _build_nc():
    import concourse.bass as bass
    import concourse.tile as tile
    from concourse import bacc, mybir

    DT = mybir.dt.float32
    BF = mybir.dt.bfloat16

    nc = bacc.Bacc(
        "TRN2", target_bir_lowering=False, debug=False, num_devices=NCORES
    )

    # x[g] for g = t*3+m : feature-map plane stacks, host-reordered
    x = nc.declare_dram_parameter("x", [TPC * 3, CCH, H, W], DT, isOutput=False)
    # block-diag conv weights: wt[(m,ic), (dy*3+dx)*96 + (m,oc)], bf16
    wt = nc.declare_dram_parameter("wt", [96, 9 * 96], BF, isOutput=False)
    # t1 weights at partitions matching their fmap: m1@0, m2@32, m0@96
    wtt1 = nc.declare_dram_parameter("wtt1", [128, 9 * 64], BF, isOutput=False)
    ident = nc.declare_dram_parameter("ident", [96, 96], BF, isOutput=False)
    g_out = nc.declare_dram_parameter("g_out", [TPC, M, M], DT, isOutput=True)

    with tile.TileContext(nc) as tc:
        with (
            tc.tile_pool(name="persist", bufs=1) as persist,
            tc.tile_pool(name="slabsA", bufs=3) as slabsA,
            tc.tile_pool(name="slabsB", bufs=3) as slabsB,
            tc.tile_pool(name="wsums", bufs=2) as wsumsA,
            tc.tile_pool(name="wsumsB", bufs=2) as wsumsB,
            tc.tile_pool(name="vt", bufs=2) as vtpool,
            tc.tile_pool(name="convps", bufs=2, space="PSUM") as convps,
            tc.tile_pool(name="vtps", bufs=2, space="PSUM") as vtps,
            tc.tile_pool(name="gramps", bufs=1, space="PSUM") as gramps,
        ):
            wt_sb = persist.tile([96, 9 * 96], BF, tag="wt")
            nc.gpsimd.dma_start(out=wt_sb[:], in_=wt.ap())
            wtt1_sb = persist.tile([128, 9 * 64], BF, tag="wtt1")
            nc.gpsimd.dma_start(out=wtt1_sb[:], in_=wtt1.ap())
            id_sb = persist.tile([96, 96], BF, tag="ident")
            nc.gpsimd.dma_start(out=id_sb[:], in_=ident.ap())

            # x-major reflect-padded pooled conv inputs (bf16)
            padA = persist.tile([128, PPIX], BF, tag="padA")  # planes 0-3
            padB = persist.tile([64, PPIX], BF, tag="padB")   # planes 4-5
            v_sb = persist.tile([96, TPC * PIX], BF, tag="v")
            g_sb = persist.tile([96, TPC * 96], DT, tag="g")

            pA3 = padA[:].rearrange("p (x y) -> p x y", x=PAD)
            pB3 = padB[:].rearrange("p (x y) -> p x y", x=PAD)

            # ---- pooling helpers ----
            def slab_dma(pas, h0, nr):
                gl, np_ = (0, 128) if pas == "A" else (4, 64)
                spool = slabsA if pas == "A" else slabsB
                slab = spool.tile([np_, nr * W], DT, tag=f"slab{pas}")
                nc.sync.dma_start(
                    out=slab[:, : nr * W],
                    in_=x.ap()[
                        gl : gl + np_ // 32, :, h0 : h0 + nr, :
                    ].rearrange("g c h w -> (g c) (h w)"),
                )
                return slab

            def slab_reduce(pas, h0, nr, slab):
                np_ = 128 if pas == "A" else 64
                pad3 = pA3 if pas == "A" else pB3
                wpool = wsumsA if pas == "A" else wsumsB
                y0 = h0 // 8
                # stage 1: w-pool 8:1, contiguous inner reduce, x-major dst
                wsum = wpool.tile([np_, HR * OUT], DT, tag=f"ws{pas}")
                nc.vector.tensor_reduce(
                    out=wsum[:, : nr * OUT].rearrange("p (x h) -> p h x", h=nr),
                    in_=slab[:, : nr * W].rearrange(
                        "p (h x b) -> p h x b", x=OUT, b=8
                    ),
                    axis=mybir.AxisListType.X,
                    op=mybir.AluOpType.add,
                )
                # stage 2: h-pool 8:1, bf16 out into padded interior rows
                with nc.allow_low_precision(
                    reason="pooled conv input in bf16 (tol 2e-2)"
                ):
                    nc.vector.tensor_reduce(
                        out=pad3[:np_, 1:25, y0 + 1 : y0 + 1 + nr // 8],
                        in_=wsum[:, : nr * OUT].rearrange(
                            "p (x hg r) -> p x hg r", hg=nr // 8, r=8
                        ),
                        axis=mybir.AxisListType.X,
                        op=mybir.AluOpType.add,
                    )

            def fix_pad0(pad3):
                # reflect pad, part 1: x-edge rows for y-cols 1..13, then
                # y-col 0 over all x
                nc.vector.tensor_copy(pad3[:, 0:1, 1:14], pad3[:, 2:3, 1:14])
                nc.vector.tensor_copy(pad3[:, 25:26, 1:14], pad3[:, 23:24, 1:14])
                nc.vector.tensor_copy(pad3[:, :, 0:1], pad3[:, :, 2:3])

            def fix_mid(pad3):
                # x-edge rows for y-cols 14..19 (unlocks the Q2 conv)
                nc.vector.tensor_copy(pad3[:, 0:1, 14:20], pad3[:, 2:3, 14:20])
                nc.vector.tensor_copy(pad3[:, 25:26, 14:20], pad3[:, 23:24, 14:20])

            def fix_pad1(pad3, c0=14):
                # reflect pad, last part: x-edge cols c0..24, then y-col 25
                nc.vector.tensor_copy(pad3[:, 0:1, c0:25], pad3[:, 2:3, c0:25])
                nc.vector.tensor_copy(pad3[:, 25:26, c0:25], pad3[:, 23:24, c0:25])
                nc.vector.tensor_copy(pad3[:, :, 25:26], pad3[:, :, 23:24])

            dydx = [(a, b) for a in range(3) for b in range(3)]
            pcs = {}

            def conv_mms(t, y0, ny):
                # conv matmuls for out rows y0..y0+ny-1 (y-major V order)
                key = (t, y0)
                pcb = convps.tile([96, 512], DT, tag="convps")
                pc = pcb[:, 0 : ny * 24]
                pcs[key] = pc
                if t == 0:
                    for i, (dy, dx) in enumerate(dydx):
                        blk = dy * 3 + dx
                        nc.tensor.matmul(
                            pc[:],
                            wt_sb[:, blk * 96 : (blk + 1) * 96],
                            pA3[:96, dx : dx + 24, y0 + dy : y0 + dy + ny
                                ].transpose([0, 2, 1]),
                            start=(i == 0), stop=(i == 8),
                        )
                else:
                    # t1: V rows (m1, m2, m0); m1+m2 fused 64-wide on padB,
                    # m0 from padA[96:128] at position (96, 64); the host
                    # permutes G[t1] back afterwards
                    for i, (dy, dx) in enumerate(dydx):
                        blk = dy * 3 + dx
                        nc.tensor.matmul(
                            pc[0:64, :],
                            wtt1_sb[0:64, blk * 64 : blk * 64 + 64],
                            pB3[:, dx : dx + 24, y0 + dy : y0 + dy + ny
                                ].transpose([0, 2, 1]),
                            start=(i == 0), stop=(i == 8),
                        )
                    for i, (dy, dx) in enumerate(dydx):
                        blk = dy * 3 + dx
                        nc.tensor.matmul(
                            pc[64:96, :],
                            wtt1_sb[96:128, blk * 64 : blk * 64 + 32],
                            pA3[96:128, dx : dx + 24, y0 + dy : y0 + dy + ny
                                ].transpose([0, 2, 1]),
                            start=(i == 0), stop=(i == 8),
                            tile_position=(96, 64),
                        )
                # stage the PSUM copy now (Act); the DVE stt comes later
                zc = vtpool.tile([96, 288], DT, tag="zcopy")
                nc.scalar.copy(zc[:, : ny * 24], pc[:])
                pcs[key] = (pc, zc)

            def relu(t, y0, ny):
                # LeakyReLU(0.2) == max(0.2*z, z)
                pc, zc = pcs[(t, y0)]
                nc.vector.scalar_tensor_tensor(
                    out=v_sb[:, t * PIX + y0 * 24 : t * PIX + (y0 + ny) * 24],
                    in0=zc[:, : ny * 24],
                    scalar=0.2,
                    in1=pc[:],
                    op0=mybir.AluOpType.mult,
                    op1=mybir.AluOpType.max,
                )

            vt_alls = {}

            def gram_chunks(t, chunks):
                if t not in vt_alls:
                    vt_all = vtpool.tile([128, 5 * 96], BF, tag=f"vtall{t}")
                    vt_alls[t] = vt_all
                vt_all = vt_alls[t]
                for c in chunks:
                    sz = 128 if c < 4 else 64
                    vslice = v_sb[:, t * PIX + c * 128 : t * PIX + c * 128 + sz]
                    ptb = vtps.tile([128, 1024], BF, tag="vtps")
                    pt = ptb[:, 0:96]
                    nc.tensor.transpose(pt[:sz, :], vslice, id_sb[:])
                    nc.scalar.copy(vt_all[:sz, c * 96 : (c + 1) * 96], pt[:sz, :])

            def gram_finish(t):
                vt_all = vt_alls[t]
                gpb = gramps.tile([96, 512], DT, tag="gram")
                gp = gpb[:, 0:96]
                for c in range(5):
                    sz = 128 if c < 4 else 64
                    nc.tensor.matmul(
                        gp[:],
                        vt_all[:sz, c * 96 : (c + 1) * 96],
                        vt_all[:sz, c * 96 : (c + 1) * 96],
                        start=(c == 0), stop=(c == 4),
                    )
                nc.scalar.copy(g_sb[:, t * 96 : (t + 1) * 96], gp[:])
                nc.gpsimd.dma_start(
                    out=g_out[t], in_=g_sb[:, t * 96 : (t + 1) * 96]
                )

            # ---- schedule: B0 first (fast DVE start); conv/relu/gram
            # stream in as their pad rows complete; relu emission is
            # deferred past the next slab's reduces so the DVE never
            # blocks on the PE ----
            slabs_seq = {
                "B0": ("B", 0, 24), "B1": ("B", 24, 24), "B2": ("B", 48, 24),
                "B3": ("B", 72, 24), "B4": ("B", 96, 24), "B5": ("B", 120, 24),
                "B6": ("B", 144, 24), "B7a": ("B", 168, 16),
                "B7b": ("B", 184, 8),
                "A0": ("A", 0, 24), "A1": ("A", 24, 24), "A2": ("A", 48, 24),
                "A3": ("A", 72, 24), "A4": ("A", 96, 24), "A5": ("A", 120, 24),
                "A6": ("A", 144, 24), "A7": ("A", 168, 24),
            }
            for step in ["B0", "A0", "A1", "B1", "A2", "B2", "A3", "B3",
                         "A4", "W0", "B4", "A5", "W0r", "B5", "W1", "A6",
                         "W1r", "A7", "W2a", "B6", "W2b", "B7a", "B7b",
                         "W3"]:
                if step == "W0":
                    fix_pad0(pA3)
                    conv_mms(0, 0, 12)
                elif step == "W0r":
                    relu(0, 0, 12)
                    gram_chunks(0, [0, 1])
                elif step == "W1":
                    fix_pad0(pB3)
                    conv_mms(1, 0, 12)
                elif step == "W1r":
                    relu(1, 0, 12)
                    gram_chunks(1, [0, 1])
                elif step == "W2a":
                    fix_pad1(pA3)
                    conv_mms(0, 12, 12)
                elif step == "W2b":
                    relu(0, 12, 12)
                    gram_chunks(0, [2, 3, 4])
                    gram_finish(0)
                    fix_mid(pB3)
                    conv_mms(1, 12, 6)
                    relu(1, 12, 6)
                    gram_chunks(1, [2])
                elif step == "W3":
                    fix_pad1(pB3, c0=20)
                    conv_mms(1, 18, 6)
                    relu(1, 18, 6)
                    gram_chunks(1, [3, 4])
                    gram_finish(1)
                else:
                    pas, h0, nr = slabs_seq[step]
                    slab_reduce(pas, h0, nr, slab_dma(pas, h0, nr))

    nc.finalize()
    return nc


def _get_nc():
    if "nc" not in _STATE:
        _STATE["nc"] = _build_nc()
    return _STATE["nc"]


def _prep_weights(W1, W2, W3):
    import ml_dtypes

    # wt[(m,ic), (dy*3+dx)*96 + 32m+oc] = W_m[oc, ic, dy, dx] / 64
    wt = np.zeros((96, 9 * 96), dtype=np.float64)
    for m, Wm in enumerate((W1, W2, W3)):
        Wm = np.asarray(Wm, np.float64) / 64.0  # [oc, ic, dy, dx]
        for dy in range(3):
            for dx in range(3):
                blk = dy * 3 + dx
                wt[
                    32 * m : 32 * m + 32,
                    blk * 96 + 32 * m : blk * 96 + 32 * m + 32,
                ] = Wm[:, :, dy, dx].T
    # wtt1 for t1 (V rows ordered m1, m2, m0): rows 0-31 = W2 ic (oc cols
    # 0-31), rows 32-63 = W3 ic (oc cols 32-63), rows 96-127 = W1 ic (oc
    # cols 0-31 of its own [32,32] slice) -- partition-aligned with fmaps
    wtt1 = np.zeros((128, 9 * 64), dtype=np.float64)
    for dy in range(3):
        for dx in range(3):
            blk = dy * 3 + dx
            w2 = np.asarray(W2, np.float64) / 64.0
            w3 = np.asarray(W3, np.float64) / 64.0
            w1 = np.asarray(W1, np.float64) / 64.0
            wtt1[0:32, blk * 64 : blk * 64 + 32] = w2[:, :, dy, dx].T
            wtt1[32:64, blk * 64 + 32 : blk * 64 + 64] = w3[:, :, dy, dx].T
            wtt1[96:128, blk * 64 : blk * 64 + 32] = w1[:, :, dy, dx].T
    bf = ml_dtypes.bfloat16
    return wt.astype(np.float32).astype(bf), wtt1.astype(np.float32).astype(bf)


def _host_loss(G):
    G = np.asarray(G, np.float64)  # [16, 96, 96]
    T = G.shape[0]
    I96 = np.eye(M)
    Me = I96[None] + ALPHA_E * G
    ld_e = 2.0 * np.log(
        np.diagonal(np.linalg.cholesky(Me), axis1=-2, axis2=-1)
    ).sum()
    blocks = np.stack(
        [G[:, 32 * c : 32 * (c + 1), 32 * c : 32 * (c + 1)] for c in range(3)]
    )  # [3, T, 32, 32]
    Mc = np.eye(32)[None, None] + ALPHA_C * blocks
    ld_c = 2.0 * np.log(
        np.diagonal(np.linalg.cholesky(Mc), axis1=-2, axis2=-1)
    ).sum()
    loss_expd = ld_e / (2.0 * T)
    loss_comp = (32.0 / M) * ld_c / (2.0 * T)
    return np.float32(loss_expd - loss_comp)


def run_device(inputs, **kw):
    """Run the bass kernel; returns (G [16,96,96], BassKernelResults)."""
    import ml_dtypes
    from concourse.bass_utils import run_bass_kernel_spmd

    nc = _get_nc()
    wt, wtt1 = _prep_weights(inputs["W1"], inputs["W2"], inputs["W3"])
    ident = np.eye(96, dtype=np.float32).astype(ml_dtypes.bfloat16)
    ms = np.asarray(inputs["ms_fea"], np.float32)
    pan = np.asarray(inputs["pan_fea"], np.float32)
    alf = np.asarray(inputs["all_fea"], np.float32)
    in_maps = []
    for i in range(NCORES):
        sl = slice(TPC * i, TPC * (i + 1))
        # x[t*3+m] = (ms,pan,alf)[m][t]
        xs = np.stack([ms[sl], pan[sl], alf[sl]], axis=1).reshape(
            TPC * 3, CCH, H, W
        )
        in_maps.append(
            {"x": np.ascontiguousarray(xs), "wt": wt, "wtt1": wtt1,
             "ident": ident}
        )
    res = run_bass_kernel_spmd(nc, in_maps, core_ids=list(range(NCORES)), **kw)
    G = np.concatenate([np.asarray(r["g_out"]) for r in res.results], axis=0)
    # odd timesteps were computed with V rows ordered (m1, m2, m0)
    perm = np.r_[64:96, 0:32, 32:64]
    G[1::2] = G[1::2][:, perm][:, :, perm]
    return G, res


def kernel(**inputs):
    G, _ = run_device(inputs)
    return _host_loss(G)
